# revision 22
# baseline (speedup 1.0000x reference)
"""Trainium2 Bass kernel for causal self-attention with T5 relative position bias.

Problem (hardcoded): B=4, T=2048, C=1024, H=16, D=64, NUM_BUCKETS=32, MAX_DISTANCE=128.
Sharding over 8 cores: core c -> (batch b=c//2, head-group hg=c%2 of 8 heads).
Each core computes qkv projection for its heads, causal attention, and a partial
output projection (its heads' rows of W_proj); host sums the two partials per batch.

v4 structure (baseline 409us -> v2 381 -> v3 ~325 -> this):
  - One software-pipelined stream of attention "superpairs": chunk-outer,
    head-PAIR inner.  The even/odd heads of a pair sit at partitions 0-63 /
    64-127, so their K=64 S matmuls land on disjoint PE row groups and run
    CONCURRENTLY (tile_position auto-derived from the base partition;
    HW-verified 216ns wall for both) -- 2x S throughput vs head-serial.
  - Pair t+1's S matmuls + EXP are emitted before pair t's AV matmuls so the
    PE never head-of-line blocks on the exp/mask chain.
  - Stage-1 (qkv) and stage-3 (proj) matmul groups are emitted as FILLER
    between pairs, paced with a terminal reserve so the PE never starves and
    the HAM clock gate keeps the array at 2.4 GHz (the phase-serial baseline
    ran most S/AV matmuls at 1.2 GHz).
  - T5 bias for far tiles (min distance >= 113: constant bucket 31) is folded
    into a second V table vf = v * exp(b31): far tiles use raw exp(S/8) with
    vf, near tiles use the ea mask/bias table with v.  No vector mask-multiply
    and no ea read for ~53% of elements; the ea table shrinks to its live
    window [384:1152) of the Toeplitz layout.
  - Softmax denominators: rowsum rows bounce through DRAM for the partition
    broadcast (a partition-strided DMA dst writes garbage on HW -- use row
    DMAs); reciprocals batch per (chunk, half) on the DVE.  The very last
    head pair is normalized via a K=1 fp16 broadcast matmul instead (PE queue
    is empty at the tail, and it skips the DRAM bounce latency).
    (reciprocal_approx_fast was tried: broken on HW, returns garbage.)
  - b_attn is always zero for this problem: evacuations are plain copies and
    the v-scatter is one strided copy per t16 tile.
  - Partial outputs yp in fp16 (host sums the two cores' partials in fp32).

On-chip layout (from baseline):
  - x, q, k transposed ([C, T]-style) so matmuls contract over partitions.
  - S computed transposed: S_T[tk, tq] = k_h^T q_h (K=64); softmax skips
    max-subtraction (logits ~ N(0,1)).
  - Superpair PSUM tile [128,1024] = [head A (512) | head B (512)], one EXP
    over both heads via a two-window strided AP.
  - Row sums from a ones-column appended to V (AV matmul M=65).
"""

import sys

sys.path.insert(0, "/opt/trn_rl_repo")

import math

import numpy as np

import concourse.bacc as bacc
import concourse.bass as bass
import concourse.mybir as mybir
import concourse.tile as tile
from concourse import bass_utils


def _ensure_axon_hooks():
    try:
        import antenv.axon_hooks  # noqa: F401
    except Exception:
        try:
            import types

            import antenv

            hooks = types.ModuleType("antenv.axon_hooks")
            hooks._hook = None
            hooks.set_axon_ntff_profile_hook = lambda h: setattr(hooks, "_hook", h)
            hooks.get_axon_ntff_profile_hook = lambda: hooks._hook
            sys.modules["antenv.axon_hooks"] = hooks
            antenv.axon_hooks = hooks
        except Exception:
            pass


_ensure_axon_hooks()

B, T, C = 4, 2048, 1024
H, D = 16, 64
NUM_BUCKETS, MAX_DISTANCE = 32, 128
HL = 8  # local heads per core
CL = HL * D  # 512 local channels
NCORES = 8

FP16 = mybir.dt.float16
FP32 = mybir.dt.float32

# ea table: live window only.  Near tiles (j >= 4c-1) read s_off in
# {384, 512} with width <= 640, so columns [384, 1152) of the full Toeplitz
# layout suffice.  EA_BASE is subtracted from s_off at use.
EA_W = 768
EA_BASE = 384
FAR_CUT = 113  # bucket(d) == 31 for all d >= 113 (fp32-exact)

NT = T // 512  # 4 tq chunks of 512
NK = T // 128  # 16 tk tiles of 128
KC = C // 128  # 8 contraction chunks for qkv
MC = CL // 128  # 4 m-chunks of local channels

# filler drain rate (groups per superpair) per chunk
DRAIN_RATE = [0.8, 0.5, 0.45, 0.45]


def _build_program():
    nc = bacc.Bacc(None, target_bir_lowering=False)

    xT = nc.dram_tensor("xT", [C, T], FP16, kind="ExternalInput")
    wq = nc.dram_tensor("wq", [C, CL], FP16, kind="ExternalInput")
    wk = nc.dram_tensor("wk", [C, CL], FP16, kind="ExternalInput")
    wv = nc.dram_tensor("wv", [C, CL], FP16, kind="ExternalInput")
    wp = nc.dram_tensor("wp", [CL, C], FP16, kind="ExternalInput")
    wexp = nc.dram_tensor("wexp", [HL, 128, EA_W], FP16, kind="ExternalInput")
    ebrow = nc.dram_tensor("ebrow", [128, HL * 65], FP16, kind="ExternalInput")
    yp = nc.dram_tensor("yp", [C, T], FP16, kind="ExternalOutput")
    rscratch = nc.dram_tensor("rscratch", [NT * HL, 512], FP16)

    with tile.TileContext(nc) as tc:
        with (
            tc.tile_pool(name="w", bufs=1) as wpool,
            tc.tile_pool(name="big", bufs=1) as bigpool,
            tc.tile_pool(name="tr", bufs=3) as tr,
            tc.tile_pool(name="sm", bufs=2) as smpool,
            tc.tile_pool(name="ev", bufs=7) as evpool,
            tc.tile_pool(name="ps", bufs=2, space="PSUM") as poolS,
            tc.tile_pool(name="pav", bufs=2, space="PSUM") as poolAV,
            tc.tile_pool(name="pf", bufs=2, space="PSUM") as poolF,
        ):
            # ---- weights / constants ----
            wq_sb = wpool.tile([128, KC, CL], FP16)
            wk_sb = wpool.tile([128, KC, CL], FP16)
            wv_sb = wpool.tile([128, KC, CL], FP16)
            wp_sb = wpool.tile([128, MC, C], FP16)
            ea_sb = wpool.tile([128, HL, EA_W], FP16)
            eb_sb = wpool.tile([128, HL * 65], FP16)
            ones_bc = wpool.tile([97, 64], FP16)
            nc.vector.memset(ones_bc[:], 1.0)
            xt_sb = bigpool.tile([128, KC, T], FP16)
            xr = xT.rearrange("(kc p) (tc t) -> p kc tc t", p=128, t=512)

            nc.sync.dma_start(out=eb_sb, in_=ebrow[:])
            wq_r = wq.rearrange("(kc p) m -> p kc m", p=128)
            wk_r = wk.rearrange("(kc p) m -> p kc m", p=128)
            wv_r = wv.rearrange("(kc p) m -> p kc m", p=128)
            for kc in range(KC):
                nc.sync.dma_start(out=xt_sb[:, kc, 0:512], in_=xr[:, kc, 0])
                nc.sync.dma_start(out=wq_sb[:, kc], in_=wq_r[:, kc])
                nc.sync.dma_start(out=wk_sb[:, kc], in_=wk_r[:, kc])
                nc.sync.dma_start(out=wv_sb[:, kc], in_=wv_r[:, kc])
            for l in range(HL):
                nc.sync.dma_start(out=ea_sb[:, l], in_=wexp[l])
            for tch in range(1, NT):
                nc.sync.dma_start(
                    out=xt_sb[:, :, tch * 512 : (tch + 1) * 512], in_=xr[:, :, tch]
                )
            nc.sync.dma_start(out=wp_sb, in_=wp.rearrange("(kc p) m -> p kc m", p=128))

            # ---- persistent activations ----
            qT_sb = bigpool.tile([128, MC, T], FP16)
            kT_sb = bigpool.tile([128, MC, T], FP16)
            v_sb = bigpool.tile([128, NK, HL * 65], FP16)  # slot l: [v(64), ones]
            vf_sb = bigpool.tile([128, NK, HL * 65], FP16)  # v * exp(b31), far tiles
            y_sb = bigpool.tile([128, MC, T], FP16)

            for l in range(HL):
                nc.vector.memset(v_sb[:, :, l * 65 + 64 : l * 65 + 65], 1.0)

            # ---- stage-1 / stage-3 matmul groups (emitted as filler) ----
            def emit_qk(which, m, tch):
                w_sb, dst = (wq_sb, qT_sb) if which == "q" else (wk_sb, kT_sb)
                tsl = slice(tch * 512, (tch + 1) * 512)
                msl = slice(m * 128, (m + 1) * 128)
                p = poolF.tile([128, 512], FP32, tag="pf")
                for kc in range(KC):
                    nc.tensor.matmul(
                        p[:], w_sb[:, kc, msl], xt_sb[:, kc, tsl],
                        start=(kc == 0), stop=(kc == KC - 1),
                    )
                nc.vector.tensor_copy(dst[:, m, tsl], p[:])

            def emit_v(t16):
                p = poolF.tile([128, 512], FP32, tag="pf")
                for kc in range(KC):
                    nc.tensor.matmul(
                        p[:], xt_sb[:, kc, t16 * 128 : (t16 + 1) * 128], wv_sb[:, kc, :],
                        start=(kc == 0), stop=(kc == KC - 1),
                    )
                base = v_sb[:, t16]
                dst = bass.AP(
                    tensor=base.tensor, offset=base.offset,
                    ap=[base.ap[0], [65, 8], [1, 64]],
                )
                src = bass.AP(
                    tensor=p.tensor, offset=p.offset,
                    ap=[p.ap[0], [64, 8], [1, 64]],
                )
                nc.vector.tensor_copy(dst, src)
                # far-tile table: v * exp(b31) columnwise (ones col becomes
                # exp(b31), giving bias-consistent rowsums)
                nc.vector.tensor_mul(
                    out=vf_sb[:, t16, :], in0=v_sb[:, t16, :], in1=eb_sb[:],
                )

            def emit_proj(tch, mo):
                tsl = slice(tch * 512, (tch + 1) * 512)
                osl = slice(mo * 128, (mo + 1) * 128)
                p = poolF.tile([128, 512], FP32, tag="pf")
                for kcm in range(MC):
                    nc.tensor.matmul(
                        p[:], wp_sb[:, kcm, osl], y_sb[:, kcm, tsl],
                        start=(kcm == 0), stop=(kcm == MC - 1),
                    )
                yo = tr.tile([128, 512], FP16, tag="yo")
                nc.scalar.activation(
                    out=yo[:], in_=p[:], func=mybir.ActivationFunctionType.Copy
                )
                nc.sync.dma_start(out=yp[osl, tsl], in_=yo[:])

            # filler queue, dependency-safe order; proj groups appended later
            fillers = []  # (key, thunk)
            for tch in range(NT):
                for t16 in range(4 * tch, 4 * tch + 4):
                    fillers.append((("v", t16), lambda t16=t16: emit_v(t16)))
                for m in range(MC):
                    fillers.append((("q", m, tch), lambda m=m, tch=tch: emit_qk("q", m, tch)))
                    fillers.append((("k", m, tch), lambda m=m, tch=tch: emit_qk("k", m, tch)))

            emitted_keys = set()
            state = {"credit": 0.0, "step": 0}

            def drain_one():
                key, thunk = fillers.pop(0)
                thunk()
                emitted_keys.add(key)

            def need(*keys):
                while fillers and not all(k in emitted_keys for k in keys):
                    drain_one()

            def drain_credit(rate):
                state["credit"] += rate
                # keep a filler reserve so the PE never starves near the end
                floor = 3 if state["step"] >= NSTEP else 3 + (NSTEP - state["step"]) // 8
                while state["credit"] >= 1.0 and len(fillers) > floor:
                    drain_one()
                    state["credit"] -= 1.0

            deferred = []  # (eligible_step, closure): normalize work is held
            # back so its upstream recip/DMA chain is done before anything it
            # emits can reach an engine queue and head-of-line stall it

            def drain_deferred(n=1):
                while n > 0 and deferred and deferred[0][0] <= state["step"]:
                    deferred.pop(0)[1]()
                    n -= 1

            # ---- the attention stream: chunk-outer, head-pair inner ----
            steps = []
            for c in range(NT):
                for hp in range(4):
                    for j in range(4 * c + 4):
                        steps.append((c, hp, j))
            NSTEP = len(steps)

            ctx = {}
            pav_tiles = {}
            rsg_tiles = {}
            yev_tiles = {}

            def emit_S(t):
                c, hp, j = steps[t]
                if j == 0:
                    if hp == 0:
                        need(("v", 4 * c + 3))
                    need(("q", hp, c), ("k", hp, c))
                off = max(0, 128 * j - 512 * c)
                far = (512 * c - 128 * j - 127) >= FAR_CUT
                w = 512 - off
                pS = poolS.tile([128, 1024], FP32, tag="pS")
                # the two heads' K=64 matmuls occupy disjoint PE row groups
                # (tile_position from base partition) and run concurrently
                nc.tensor.matmul(
                    pS[:, off:512],
                    kT_sb[0:64, hp, j * 128 : (j + 1) * 128],
                    qT_sb[0:64, hp, c * 512 + off : (c + 1) * 512],
                    start=True, stop=True,
                )
                nc.tensor.matmul(
                    pS[:, 512 + off : 1024],
                    kT_sb[64:128, hp, j * 128 : (j + 1) * 128],
                    qT_sb[64:128, hp, c * 512 + off : (c + 1) * 512],
                    start=True, stop=True,
                )

                def win(ap_t):
                    return bass.AP(
                        tensor=ap_t.tensor, offset=ap_t.offset + off,
                        ap=[ap_t.ap[0], [512, 2], [1, w]],
                    )

                pm2 = tr.tile([128, 1024], FP16, tag="pm")
                if far:
                    nc.scalar.activation(
                        out=win(pm2), in_=win(pS),
                        func=mybir.ActivationFunctionType.Exp,
                        scale=1.0 / math.sqrt(D),
                    )
                else:
                    p2 = tr.tile([128, 1024], FP16, tag="p")
                    nc.scalar.activation(
                        out=win(p2), in_=win(pS),
                        func=mybir.ActivationFunctionType.Exp,
                        scale=1.0 / math.sqrt(D),
                    )
                    s_off = 512 * c - 128 * j + 384 + off
                    ea_h = ea_sb[:, 2 * hp]
                    ea_ap = bass.AP(
                        tensor=ea_h.tensor,
                        offset=ea_h.offset + (s_off - EA_BASE),
                        ap=[ea_h.ap[0], [EA_W, 2], [1, w]],
                    )
                    nc.vector.tensor_mul(out=win(pm2), in0=win(p2), in1=ea_ap)
                ctx[t] = (c, hp, j, pm2, off, far)

            def emit_AV(t):
                c, hp, j, pm2, off, far = ctx.pop(t)
                nj = 4 * c + 4
                vtab = vf_sb if far else v_sb
                for h in range(2):
                    l = 2 * hp + h
                    if j == 0:
                        pav_tiles[(c, l)] = poolAV.tile(
                            [65, 512], FP32, tag="pav", name="pav"
                        )
                    pav = pav_tiles[(c, l)]
                    nc.tensor.matmul(
                        pav[:, off:512],
                        vtab[:, j, l * 65 : l * 65 + 65],
                        pm2[:, 512 * h + off : 512 * h + 512],
                        start=(j == 0),
                        stop=(j == nj - 1),
                    )
                if j == nj - 1:
                    for h in range(2):
                        post_head_chunk(c, 2 * hp + h)

            def post_head_chunk(c, l):
                pav = pav_tiles.pop((c, l))
                yev = evpool.tile([128, 512], FP32, tag="yev")
                nc.vector.tensor_copy(yev[0:65, :], pav[0:65, :])
                yev_tiles[(c, l)] = yev
                if c == NT - 1 and l >= 6:
                    tail_norm(l)
                    return
                if c == NT - 1 and l >= 4:
                    # chunk 3 heads 4,5: finish early so the recip runs during
                    # the final head pair
                    if l == 4:
                        rsg_tiles[(c, 1)] = smpool.tile(
                            [4, 512], FP32, tag="rsg", name="rsg32"
                        )
                    nc.sync.dma_start(
                        out=rsg_tiles[(c, 1)][l - 4 : l - 3, :], in_=yev[64:65, :]
                    )
                    if l == 5:
                        finish_half(c, 1, nrows=2)
                    return
                half = l // 4
                if l % 4 == 0:
                    rsg_tiles[(c, half)] = smpool.tile(
                        [4, 512], FP32, tag="rsg", name="rsg32"
                    )
                nc.sync.dma_start(
                    out=rsg_tiles[(c, half)][l % 4 : l % 4 + 1, :], in_=yev[64:65, :]
                )
                if l % 4 == 3:
                    finish_half(c, half)

            def finish_half(c, half, nrows=4):
                # reciprocal cost is ~6.4ns per free element regardless of
                # partition count; emit it in 128-column pieces drained one
                # per superpair so the DVE queue never blocks ~3.3us at once
                rsg = rsg_tiles.pop((c, half))
                rec32 = smpool.tile([4, 512], FP32, tag="rec32")
                rec16 = smpool.tile([4, 512], FP16, tag="rec16")
                s0 = state["step"]

                def piece(q):
                    cs = slice(q * 128, (q + 1) * 128)
                    nc.vector.reciprocal(out=rec32[0:nrows, cs], in_=rsg[0:nrows, cs])

                def flush():
                    nc.vector.tensor_copy(rec16[0:nrows, :], rec32[0:nrows, :])
                    r0 = c * HL + half * 4
                    nc.sync.dma_start(
                        out=rscratch[r0 : r0 + nrows], in_=rec16[0:nrows, :]
                    )

                for q in range(4):
                    deferred.append((s0 + 1 + q, lambda q=q: piece(q)))
                deferred.append((s0 + 5, lambda: flush()))
                elig = s0 + 7
                for l in range(half * 4, half * 4 + nrows):
                    deferred.append(
                        (elig, lambda c=c, l=l: normalize_bounce(c, l))
                    )
                if half == 1 and nrows == 4:
                    deferred.append((elig, lambda c=c: push_proj(c)))

            tail_state = {}

            def tail_norm(l):
                # heads 6,7 of the last chunk: batched recip + in-SBUF
                # broadcast via K=1 fp16 matmuls (PE queue is empty here)
                c = NT - 1
                if l == 6:
                    trsg = smpool.tile([2, 512], FP32, tag="rsg", name="trsg")
                    tail_state["trsg"] = trsg
                    nc.sync.dma_start(out=trsg[0:1, :], in_=yev_tiles[(c, 6)][64:65, :])
                    return
                trsg = tail_state["trsg"]
                nc.sync.dma_start(out=trsg[1:2, :], in_=yev_tiles[(c, 7)][64:65, :])
                while fillers:
                    drain_one()  # reserved groups keep the PE warm here
                trec = smpool.tile([33, 512], FP32, tag="rec32", name="trec")
                nc.vector.reciprocal(out=trec[0:2, :], in_=trsg[0:2, :])
                trec16 = smpool.tile([33, 512], FP16, tag="rec16", name="trec16")
                nc.vector.tensor_copy(trec16[0:2, :], trec[0:2, :])
                nc.sync.dma_start(out=trec16[32:33, :], in_=trec16[1:2, :])
                normalize(c, 6, trec16[0:1, :], bp=0)
                normalize(c, 7, trec16[32:33, :], bp=32)
                push_proj(c)

            def push_proj(c):
                for mo in range(C // 128):
                    fillers.append((("proj", c, mo), lambda c=c, mo=mo: emit_proj(c, mo)))

            def normalize_bounce(c, l):
                srow = rscratch[c * HL + l]
                bc_sb = smpool.tile([64, 512], FP16, tag="bcsb")
                nc.sync.dma_start(
                    out=bc_sb[:],
                    in_=bass.AP(
                        tensor=srow.tensor, offset=srow.offset,
                        ap=[[0, 64], [1, 512]],
                    ),
                )
                finish_norm(c, l, bc_sb[:])

            def normalize(c, l, rrow, bp=0):
                bcp = poolF.tile([64, 512], FP32, tag="pf", name="bcp")
                nc.tensor.matmul(
                    bcp[:], ones_bc[bp : bp + 1, :], rrow, start=True, stop=True
                )
                finish_norm(c, l, bcp[:])

            def finish_norm(c, l, scale_ap):
                mq = l // 2
                fullq = slice(c * 512, (c + 1) * 512)
                yev = yev_tiles.pop((c, l))
                if l % 2 == 0:
                    nc.vector.tensor_mul(
                        out=y_sb[0:64, mq, fullq], in0=yev[0:64, :], in1=scale_ap,
                    )
                else:
                    ytmp = smpool.tile([64, 512], FP16, tag="ytmp")
                    nc.vector.tensor_mul(out=ytmp[:], in0=yev[0:64, :], in1=scale_ap)
                    nc.sync.dma_start(out=y_sb[64:128, mq, fullq], in_=ytmp[:])

            # ---- run the pipelined stream ----
            emit_S(0)
            for t in range(1, NSTEP):
                state["step"] = t
                emit_S(t)
                emit_AV(t - 1)
                drain_deferred(1)
                drain_credit(DRAIN_RATE[steps[t][0]])
            state["step"] = NSTEP + 99
            emit_AV(NSTEP - 1)
            drain_deferred(99)
            while fillers:
                drain_one()

    nc.compile()
    return nc


_NC = None
LAST_RESULTS = None


def _get_program():
    global _NC
    if _NC is None:
        _NC = _build_program()
    return _NC


def _rel_bias_buckets():
    """bucket(d) for d = q - k in [0, T): exact float32 replica of the reference."""
    d = np.arange(T)
    max_exact = NUM_BUCKETS // 2
    rpf = d.astype(np.float32) / np.float32(max_exact) + np.float32(1e-10)
    val = (
        np.log(rpf)
        / np.float32(math.log(MAX_DISTANCE / max_exact))
        * np.float32(NUM_BUCKETS - max_exact)
    )
    large = max_exact + val.astype(np.int32)
    large = np.minimum(large, NUM_BUCKETS - 1)
    return np.where(d < max_exact, d, large)


def _make_in_maps(x, W_attn, b_attn, W_proj, rel_emb):
    buckets = _rel_bias_buckets()  # [T]
    assert buckets[FAR_CUT - 1] < NUM_BUCKETS - 1
    assert buckets[FAR_CUT] == NUM_BUCKETS - 1
    bias_by_dist = rel_emb[buckets, :]  # [T, H] fp32
    # vec[h, j] = exp(bias[j - 511]) for j >= 511 else 0   (j - 511 = distance d)
    vec = np.zeros((H, 2432 + 127), dtype=np.float32)
    vec[:, 511 : 511 + T] = np.exp(bias_by_dist.T)
    vec = vec.astype(np.float16)
    # per-head Toeplitz table A[h, p, x] = vec[h, x - p + 127], live window only
    sw = np.lib.stride_tricks.sliding_window_view(vec, 2432, axis=1)  # [H, 128, 2432]
    wexp_all = np.ascontiguousarray(sw[:, ::-1, EA_BASE : EA_BASE + EA_W])

    # ebrow: exp(b31) per local head, replicated over the 65 slot columns
    eb = np.exp(rel_emb[NUM_BUCKETS - 1, :].astype(np.float32))  # [H]
    ebrow_all = np.broadcast_to(
        np.repeat(eb, 65)[None, :], (128, H * 65)
    ).astype(np.float16)

    in_maps = []
    for core in range(NCORES):
        b, hg = core // 2, core % 2
        csl = slice(hg * CL, (hg + 1) * CL)
        in_maps.append(
            {
                "xT": np.ascontiguousarray(x[b].T).astype(np.float16),
                "wq": np.ascontiguousarray(W_attn[csl, :].T).astype(np.float16),
                "wk": np.ascontiguousarray(W_attn[C + hg * CL : C + (hg + 1) * CL, :].T).astype(np.float16),
                "wv": np.ascontiguousarray(W_attn[2 * C + hg * CL : 2 * C + (hg + 1) * CL, :].T).astype(np.float16),
                "wp": np.ascontiguousarray(W_proj[:, csl].T).astype(np.float16),
                "wexp": np.ascontiguousarray(wexp_all[hg * HL : (hg + 1) * HL]),
                "ebrow": np.ascontiguousarray(
                    ebrow_all[:, hg * HL * 65 : (hg + 1) * HL * 65]
                ),
            }
        )
    return in_maps


def kernel(x, W_attn, b_attn, W_proj, b_proj, rel_emb):
    x = np.asarray(x)
    W_attn = np.asarray(W_attn)
    b_attn = np.asarray(b_attn)
    W_proj = np.asarray(W_proj)
    b_proj = np.asarray(b_proj)
    rel_emb = np.asarray(rel_emb)
    assert not b_attn.any(), "kernel specialised for zero attn bias"

    in_maps = _make_in_maps(x, W_attn, b_attn, W_proj, rel_emb)
    nc = _get_program()
    res = bass_utils.run_bass_kernel_spmd(nc, in_maps, core_ids=list(range(NCORES)))
    global LAST_RESULTS
    LAST_RESULTS = res

    y = np.empty((B, T, C), dtype=np.float32)
    for b in range(B):
        ypT = res.results[2 * b]["yp"].astype(np.float32) + res.results[2 * b + 1][
            "yp"
        ].astype(np.float32)
        y[b] = ypT.T + b_proj[None, :].astype(np.float32)
    return y


# revision 23
# speedup vs baseline: 1.0162x; 1.0162x over previous
"""Trainium2 Bass kernel for causal self-attention with T5 relative position bias.

Problem (hardcoded): B=4, T=2048, C=1024, H=16, D=64, NUM_BUCKETS=32, MAX_DISTANCE=128.
Sharding over 8 cores: core c -> (batch b=c//2, head-group hg=c%2 of 8 heads).
Each core computes qkv projection for its heads, causal attention, and a partial
output projection (its heads' rows of W_proj); host sums the two partials per batch.

v4 structure (baseline 409us -> v2 381 -> v3 ~325 -> this):
  - One software-pipelined stream of attention "superpairs": chunk-outer,
    head-PAIR inner.  The even/odd heads of a pair sit at partitions 0-63 /
    64-127, so their K=64 S matmuls land on disjoint PE row groups and run
    CONCURRENTLY (tile_position auto-derived from the base partition;
    HW-verified 216ns wall for both) -- 2x S throughput vs head-serial.
  - Pair t+1's S matmuls + EXP are emitted before pair t's AV matmuls so the
    PE never head-of-line blocks on the exp/mask chain.
  - Stage-1 (qkv) and stage-3 (proj) matmul groups are emitted as FILLER
    between pairs, paced with a terminal reserve so the PE never starves and
    the HAM clock gate keeps the array at 2.4 GHz (the phase-serial baseline
    ran most S/AV matmuls at 1.2 GHz).
  - T5 bias for far tiles (min distance >= 113: constant bucket 31) is folded
    into a second V table vf = v * exp(b31): far tiles use raw exp(S/8) with
    vf, near tiles use the ea mask/bias table with v.  No vector mask-multiply
    and no ea read for ~53% of elements; the ea table shrinks to its live
    window [384:1152) of the Toeplitz layout.
  - Softmax denominators: rowsum rows bounce through DRAM for the partition
    broadcast (a partition-strided DMA dst writes garbage on HW -- use row
    DMAs); reciprocals batch per (chunk, half) on the DVE.  The very last
    head pair is normalized via a K=1 fp16 broadcast matmul instead (PE queue
    is empty at the tail, and it skips the DRAM bounce latency).
    (reciprocal_approx_fast was tried: broken on HW, returns garbage.)
  - b_attn is always zero for this problem: evacuations are plain copies and
    the v-scatter is one strided copy per t16 tile.
  - Partial outputs yp in fp16 (host sums the two cores' partials in fp32).

On-chip layout (from baseline):
  - x, q, k transposed ([C, T]-style) so matmuls contract over partitions.
  - S computed transposed: S_T[tk, tq] = k_h^T q_h (K=64); softmax skips
    max-subtraction (logits ~ N(0,1)).
  - Superpair PSUM tile [128,1024] = [head A (512) | head B (512)], one EXP
    over both heads via a two-window strided AP.
  - Row sums from a ones-column appended to V (AV matmul M=65).
"""

import sys

sys.path.insert(0, "/opt/trn_rl_repo")

import math

import numpy as np

import concourse.bacc as bacc
import concourse.bass as bass
import concourse.mybir as mybir
import concourse.tile as tile
from concourse import bass_utils


def _ensure_axon_hooks():
    try:
        import antenv.axon_hooks  # noqa: F401
    except Exception:
        try:
            import types

            import antenv

            hooks = types.ModuleType("antenv.axon_hooks")
            hooks._hook = None
            hooks.set_axon_ntff_profile_hook = lambda h: setattr(hooks, "_hook", h)
            hooks.get_axon_ntff_profile_hook = lambda: hooks._hook
            sys.modules["antenv.axon_hooks"] = hooks
            antenv.axon_hooks = hooks
        except Exception:
            pass


_ensure_axon_hooks()

B, T, C = 4, 2048, 1024
H, D = 16, 64
NUM_BUCKETS, MAX_DISTANCE = 32, 128
HL = 8  # local heads per core
CL = HL * D  # 512 local channels
NCORES = 8

FP16 = mybir.dt.float16
FP32 = mybir.dt.float32

# ea table: live window only.  Near tiles (j >= 4c-1) read s_off in
# {384, 512} with width <= 640, so columns [384, 1152) of the full Toeplitz
# layout suffice.  EA_BASE is subtracted from s_off at use.
EA_W = 768
EA_BASE = 384
FAR_CUT = 113  # bucket(d) == 31 for all d >= 113 (fp32-exact)

NT = T // 512  # 4 tq chunks of 512
NK = T // 128  # 16 tk tiles of 128
KC = C // 128  # 8 contraction chunks for qkv
MC = CL // 128  # 4 m-chunks of local channels

# filler drain rate (groups per superpair) per chunk
DRAIN_RATE = [0.8, 0.5, 0.45, 0.45]


def _build_program():
    nc = bacc.Bacc(None, target_bir_lowering=False)

    xT = nc.dram_tensor("xT", [C, T], FP16, kind="ExternalInput")
    wq = nc.dram_tensor("wq", [C, CL], FP16, kind="ExternalInput")
    wk = nc.dram_tensor("wk", [C, CL], FP16, kind="ExternalInput")
    wv = nc.dram_tensor("wv", [C, CL], FP16, kind="ExternalInput")
    wp = nc.dram_tensor("wp", [CL, C], FP16, kind="ExternalInput")
    wexp = nc.dram_tensor("wexp", [HL, 128, EA_W], FP16, kind="ExternalInput")
    ebrow = nc.dram_tensor("ebrow", [128, HL * 65], FP16, kind="ExternalInput")
    yp = nc.dram_tensor("yp", [C, T], FP16, kind="ExternalOutput")
    rscratch = nc.dram_tensor("rscratch", [NT * HL, 512], FP16)

    with tile.TileContext(nc) as tc:
        with (
            tc.tile_pool(name="w", bufs=1) as wpool,
            tc.tile_pool(name="big", bufs=1) as bigpool,
            tc.tile_pool(name="tr", bufs=3) as tr,
            tc.tile_pool(name="sm", bufs=2) as smpool,
            tc.tile_pool(name="ev", bufs=7) as evpool,
            tc.tile_pool(name="ps", bufs=2, space="PSUM") as poolS,
            tc.tile_pool(name="pav", bufs=2, space="PSUM") as poolAV,
            tc.tile_pool(name="pf", bufs=2, space="PSUM") as poolF,
        ):
            # ---- weights / constants ----
            wq_sb = wpool.tile([128, KC, CL], FP16)
            wk_sb = wpool.tile([128, KC, CL], FP16)
            wv_sb = wpool.tile([128, KC, CL], FP16)
            wp_sb = wpool.tile([128, MC, C], FP16)
            ea_sb = wpool.tile([128, HL, EA_W], FP16)
            eb_sb = wpool.tile([128, HL * 65], FP16)
            ones_bc = wpool.tile([97, 64], FP16)
            nc.vector.memset(ones_bc[:], 1.0)
            xt_sb = bigpool.tile([128, KC, T], FP16)
            xr = xT.rearrange("(kc p) (tc t) -> p kc tc t", p=128, t=512)

            nc.sync.dma_start(out=eb_sb, in_=ebrow[:])
            wq_r = wq.rearrange("(kc p) m -> p kc m", p=128)
            wk_r = wk.rearrange("(kc p) m -> p kc m", p=128)
            wv_r = wv.rearrange("(kc p) m -> p kc m", p=128)
            for kc in range(KC):
                nc.sync.dma_start(out=xt_sb[:, kc, 0:512], in_=xr[:, kc, 0])
                nc.sync.dma_start(out=wq_sb[:, kc], in_=wq_r[:, kc])
                nc.sync.dma_start(out=wk_sb[:, kc], in_=wk_r[:, kc])
                nc.sync.dma_start(out=wv_sb[:, kc], in_=wv_r[:, kc])
            for l in range(HL):
                nc.sync.dma_start(out=ea_sb[:, l], in_=wexp[l])
            for tch in range(1, NT):
                nc.sync.dma_start(
                    out=xt_sb[:, :, tch * 512 : (tch + 1) * 512], in_=xr[:, :, tch]
                )
            nc.sync.dma_start(out=wp_sb, in_=wp.rearrange("(kc p) m -> p kc m", p=128))

            # ---- persistent activations ----
            qT_sb = bigpool.tile([128, MC, T], FP16)
            kT_sb = bigpool.tile([128, MC, T], FP16)
            v_sb = bigpool.tile([128, NK, HL * 65], FP16)  # slot l: [v(64), ones]
            vf_sb = bigpool.tile([128, NK, HL * 65], FP16)  # v * exp(b31), far tiles
            y_sb = bigpool.tile([128, MC, T], FP16)

            for l in range(HL):
                nc.vector.memset(v_sb[:, :, l * 65 + 64 : l * 65 + 65], 1.0)

            # ---- stage-1 / stage-3 matmul groups (emitted as filler) ----
            def emit_qk(which, m, tch):
                w_sb, dst = (wq_sb, qT_sb) if which == "q" else (wk_sb, kT_sb)
                tsl = slice(tch * 512, (tch + 1) * 512)
                msl = slice(m * 128, (m + 1) * 128)
                p = poolF.tile([128, 512], FP32, tag="pf")
                for kc in range(KC):
                    nc.tensor.matmul(
                        p[:], w_sb[:, kc, msl], xt_sb[:, kc, tsl],
                        start=(kc == 0), stop=(kc == KC - 1),
                    )
                nc.vector.tensor_copy(dst[:, m, tsl], p[:])

            def emit_v(t16):
                p = poolF.tile([128, 512], FP32, tag="pf")
                for kc in range(KC):
                    nc.tensor.matmul(
                        p[:], xt_sb[:, kc, t16 * 128 : (t16 + 1) * 128], wv_sb[:, kc, :],
                        start=(kc == 0), stop=(kc == KC - 1),
                    )
                base = v_sb[:, t16]
                dst = bass.AP(
                    tensor=base.tensor, offset=base.offset,
                    ap=[base.ap[0], [65, 8], [1, 64]],
                )
                src = bass.AP(
                    tensor=p.tensor, offset=p.offset,
                    ap=[p.ap[0], [64, 8], [1, 64]],
                )
                nc.vector.tensor_copy(dst, src)
                # far-tile table: v * exp(b31) columnwise (ones col becomes
                # exp(b31), giving bias-consistent rowsums)
                nc.vector.tensor_mul(
                    out=vf_sb[:, t16, :], in0=v_sb[:, t16, :], in1=eb_sb[:],
                )

            def emit_proj(tch, mo):
                tsl = slice(tch * 512, (tch + 1) * 512)
                osl = slice(mo * 128, (mo + 1) * 128)
                p = poolF.tile([128, 512], FP32, tag="pf")
                for kcm in range(MC):
                    nc.tensor.matmul(
                        p[:], wp_sb[:, kcm, osl], y_sb[:, kcm, tsl],
                        start=(kcm == 0), stop=(kcm == MC - 1),
                    )
                yo = tr.tile([128, 512], FP16, tag="yo")
                nc.vector.tensor_copy(yo[:], p[:])
                nc.sync.dma_start(out=yp[osl, tsl], in_=yo[:])

            # filler queue, dependency-safe order; proj groups appended later
            fillers = []  # (key, thunk)
            for tch in range(NT):
                for t16 in range(4 * tch, 4 * tch + 4):
                    fillers.append((("v", t16), lambda t16=t16: emit_v(t16)))
                for m in range(MC):
                    fillers.append((("q", m, tch), lambda m=m, tch=tch: emit_qk("q", m, tch)))
                    fillers.append((("k", m, tch), lambda m=m, tch=tch: emit_qk("k", m, tch)))

            emitted_keys = set()
            state = {"credit": 0.0, "step": 0}

            def drain_one():
                key, thunk = fillers.pop(0)
                thunk()
                emitted_keys.add(key)

            def need(*keys):
                while fillers and not all(k in emitted_keys for k in keys):
                    drain_one()

            def drain_credit(rate):
                state["credit"] += rate
                # keep a filler reserve so the PE never starves near the end
                floor = 3 if state["step"] >= NSTEP else 3 + (NSTEP - state["step"]) // 8
                while state["credit"] >= 1.0 and len(fillers) > floor:
                    drain_one()
                    state["credit"] -= 1.0

            deferred = []  # (eligible_step, closure): normalize work is held
            # back so its upstream recip/DMA chain is done before anything it
            # emits can reach an engine queue and head-of-line stall it

            def drain_deferred(n=1):
                while n > 0 and deferred and deferred[0][0] <= state["step"]:
                    deferred.pop(0)[1]()
                    n -= 1

            # ---- the attention stream: chunk-outer, head-pair inner ----
            steps = []
            for c in range(NT):
                for hp in range(4):
                    for j in range(4 * c + 4):
                        steps.append((c, hp, j))
            NSTEP = len(steps)

            ctx = {}
            pav_tiles = {}
            rsg_tiles = {}
            yev_tiles = {}

            def emit_S(t):
                c, hp, j = steps[t]
                if j == 0:
                    if hp == 0:
                        need(("v", 4 * c + 3))
                    need(("q", hp, c), ("k", hp, c))
                off = max(0, 128 * j - 512 * c)
                far = (512 * c - 128 * j - 127) >= FAR_CUT
                w = 512 - off
                pS = poolS.tile([128, 1024], FP32, tag="pS")
                # the two heads' K=64 matmuls occupy disjoint PE row groups
                # (tile_position from base partition) and run concurrently
                nc.tensor.matmul(
                    pS[:, off:512],
                    kT_sb[0:64, hp, j * 128 : (j + 1) * 128],
                    qT_sb[0:64, hp, c * 512 + off : (c + 1) * 512],
                    start=True, stop=True,
                )
                nc.tensor.matmul(
                    pS[:, 512 + off : 1024],
                    kT_sb[64:128, hp, j * 128 : (j + 1) * 128],
                    qT_sb[64:128, hp, c * 512 + off : (c + 1) * 512],
                    start=True, stop=True,
                )

                def win(ap_t):
                    return bass.AP(
                        tensor=ap_t.tensor, offset=ap_t.offset + off,
                        ap=[ap_t.ap[0], [512, 2], [1, w]],
                    )

                pm2 = tr.tile([128, 1024], FP16, tag="pm")
                if far:
                    nc.scalar.activation(
                        out=win(pm2), in_=win(pS),
                        func=mybir.ActivationFunctionType.Exp,
                        scale=1.0 / math.sqrt(D),
                    )
                else:
                    p2 = tr.tile([128, 1024], FP16, tag="p")
                    nc.scalar.activation(
                        out=win(p2), in_=win(pS),
                        func=mybir.ActivationFunctionType.Exp,
                        scale=1.0 / math.sqrt(D),
                    )
                    s_off = 512 * c - 128 * j + 384 + off
                    ea_h = ea_sb[:, 2 * hp]
                    ea_ap = bass.AP(
                        tensor=ea_h.tensor,
                        offset=ea_h.offset + (s_off - EA_BASE),
                        ap=[ea_h.ap[0], [EA_W, 2], [1, w]],
                    )
                    nc.vector.tensor_mul(out=win(pm2), in0=win(p2), in1=ea_ap)
                ctx[t] = (c, hp, j, pm2, off, far)

            def emit_AV(t):
                c, hp, j, pm2, off, far = ctx.pop(t)
                nj = 4 * c + 4
                vtab = vf_sb if far else v_sb
                for h in range(2):
                    l = 2 * hp + h
                    if j == 0:
                        pav_tiles[(c, l)] = poolAV.tile(
                            [65, 512], FP32, tag="pav", name="pav"
                        )
                    pav = pav_tiles[(c, l)]
                    nc.tensor.matmul(
                        pav[:, off:512],
                        vtab[:, j, l * 65 : l * 65 + 65],
                        pm2[:, 512 * h + off : 512 * h + 512],
                        start=(j == 0),
                        stop=(j == nj - 1),
                    )
                if j == nj - 1:
                    for h in range(2):
                        post_head_chunk(c, 2 * hp + h)

            def post_head_chunk(c, l):
                pav = pav_tiles.pop((c, l))
                yev = evpool.tile([128, 512], FP32, tag="yev")
                nc.vector.tensor_copy(yev[0:65, :], pav[0:65, :])
                yev_tiles[(c, l)] = yev
                if c == NT - 1 and l >= 6:
                    tail_norm(l)
                    return
                if c == NT - 1 and l >= 4:
                    # chunk 3 heads 4,5: finish early so the recip runs during
                    # the final head pair
                    if l == 4:
                        rsg_tiles[(c, 1)] = smpool.tile(
                            [4, 512], FP32, tag="rsg", name="rsg32"
                        )
                    nc.sync.dma_start(
                        out=rsg_tiles[(c, 1)][l - 4 : l - 3, :], in_=yev[64:65, :]
                    )
                    if l == 5:
                        finish_half(c, 1, nrows=2)
                    return
                half = l // 4
                if l % 4 == 0:
                    rsg_tiles[(c, half)] = smpool.tile(
                        [4, 512], FP32, tag="rsg", name="rsg32"
                    )
                nc.sync.dma_start(
                    out=rsg_tiles[(c, half)][l % 4 : l % 4 + 1, :], in_=yev[64:65, :]
                )
                if l % 4 == 3:
                    finish_half(c, half)

            def finish_half(c, half, nrows=4):
                # reciprocal cost is ~6.4ns per free element regardless of
                # partition count; emit it in 128-column pieces drained one
                # per superpair so the DVE queue never blocks ~3.3us at once
                rsg = rsg_tiles.pop((c, half))
                rec32 = smpool.tile([4, 512], FP32, tag="rec32")
                rec16 = smpool.tile([4, 512], FP16, tag="rec16")
                s0 = state["step"]

                def piece(q):
                    cs = slice(q * 128, (q + 1) * 128)
                    nc.vector.reciprocal(out=rec32[0:nrows, cs], in_=rsg[0:nrows, cs])

                def flush():
                    nc.vector.tensor_copy(rec16[0:nrows, :], rec32[0:nrows, :])
                    r0 = c * HL + half * 4
                    nc.sync.dma_start(
                        out=rscratch[r0 : r0 + nrows], in_=rec16[0:nrows, :]
                    )

                for q in range(4):
                    deferred.append((s0 + 1 + q, lambda q=q: piece(q)))
                deferred.append((s0 + 5, lambda: flush()))
                elig = s0 + 7
                for l in range(half * 4, half * 4 + nrows):
                    deferred.append(
                        (elig, lambda c=c, l=l: normalize_bounce(c, l))
                    )
                if half == 1 and nrows == 4:
                    deferred.append((elig, lambda c=c: push_proj(c)))

            tail_state = {}

            def tail_norm(l):
                # heads 6,7 of the last chunk: batched recip + in-SBUF
                # broadcast via K=1 fp16 matmuls (PE queue is empty here)
                c = NT - 1
                if l == 6:
                    trsg = smpool.tile([2, 512], FP32, tag="rsg", name="trsg")
                    tail_state["trsg"] = trsg
                    nc.sync.dma_start(out=trsg[0:1, :], in_=yev_tiles[(c, 6)][64:65, :])
                    return
                trsg = tail_state["trsg"]
                nc.sync.dma_start(out=trsg[1:2, :], in_=yev_tiles[(c, 7)][64:65, :])
                while fillers:
                    drain_one()  # reserved groups keep the PE warm here
                trec = smpool.tile([33, 512], FP32, tag="rec32", name="trec")
                nc.vector.reciprocal(out=trec[0:2, :], in_=trsg[0:2, :])
                trec16 = smpool.tile([33, 512], FP16, tag="rec16", name="trec16")
                nc.vector.tensor_copy(trec16[0:2, :], trec[0:2, :])
                nc.sync.dma_start(out=trec16[32:33, :], in_=trec16[1:2, :])
                normalize(c, 6, trec16[0:1, :], bp=0)
                normalize(c, 7, trec16[32:33, :], bp=32)
                push_proj(c)

            def push_proj(c):
                for mo in range(C // 128):
                    fillers.append((("proj", c, mo), lambda c=c, mo=mo: emit_proj(c, mo)))

            def normalize_bounce(c, l):
                srow = rscratch[c * HL + l]
                bc_sb = smpool.tile([64, 512], FP16, tag="bcsb")
                nc.sync.dma_start(
                    out=bc_sb[:],
                    in_=bass.AP(
                        tensor=srow.tensor, offset=srow.offset,
                        ap=[[0, 64], [1, 512]],
                    ),
                )
                finish_norm(c, l, bc_sb[:])

            def normalize(c, l, rrow, bp=0):
                bcp = poolF.tile([64, 512], FP32, tag="pf", name="bcp")
                nc.tensor.matmul(
                    bcp[:], ones_bc[bp : bp + 1, :], rrow, start=True, stop=True
                )
                finish_norm(c, l, bcp[:])

            def finish_norm(c, l, scale_ap):
                mq = l // 2
                fullq = slice(c * 512, (c + 1) * 512)
                yev = yev_tiles.pop((c, l))
                if l % 2 == 0:
                    nc.vector.tensor_mul(
                        out=y_sb[0:64, mq, fullq], in0=yev[0:64, :], in1=scale_ap,
                    )
                else:
                    ytmp = smpool.tile([64, 512], FP16, tag="ytmp")
                    nc.vector.tensor_mul(out=ytmp[:], in0=yev[0:64, :], in1=scale_ap)
                    nc.sync.dma_start(out=y_sb[64:128, mq, fullq], in_=ytmp[:])

            # ---- run the pipelined stream ----
            emit_S(0)
            for t in range(1, NSTEP):
                state["step"] = t
                emit_S(t)
                emit_AV(t - 1)
                drain_deferred(1)
                drain_credit(DRAIN_RATE[steps[t][0]])
            state["step"] = NSTEP + 99
            emit_AV(NSTEP - 1)
            drain_deferred(99)
            while fillers:
                drain_one()

    nc.compile()
    return nc


_NC = None
LAST_RESULTS = None


def _get_program():
    global _NC
    if _NC is None:
        _NC = _build_program()
    return _NC


def _rel_bias_buckets():
    """bucket(d) for d = q - k in [0, T): exact float32 replica of the reference."""
    d = np.arange(T)
    max_exact = NUM_BUCKETS // 2
    rpf = d.astype(np.float32) / np.float32(max_exact) + np.float32(1e-10)
    val = (
        np.log(rpf)
        / np.float32(math.log(MAX_DISTANCE / max_exact))
        * np.float32(NUM_BUCKETS - max_exact)
    )
    large = max_exact + val.astype(np.int32)
    large = np.minimum(large, NUM_BUCKETS - 1)
    return np.where(d < max_exact, d, large)


def _make_in_maps(x, W_attn, b_attn, W_proj, rel_emb):
    buckets = _rel_bias_buckets()  # [T]
    assert buckets[FAR_CUT - 1] < NUM_BUCKETS - 1
    assert buckets[FAR_CUT] == NUM_BUCKETS - 1
    bias_by_dist = rel_emb[buckets, :]  # [T, H] fp32
    # vec[h, j] = exp(bias[j - 511]) for j >= 511 else 0   (j - 511 = distance d)
    vec = np.zeros((H, 2432 + 127), dtype=np.float32)
    vec[:, 511 : 511 + T] = np.exp(bias_by_dist.T)
    vec = vec.astype(np.float16)
    # per-head Toeplitz table A[h, p, x] = vec[h, x - p + 127], live window only
    sw = np.lib.stride_tricks.sliding_window_view(vec, 2432, axis=1)  # [H, 128, 2432]
    wexp_all = np.ascontiguousarray(sw[:, ::-1, EA_BASE : EA_BASE + EA_W])

    # ebrow: exp(b31) per local head, replicated over the 65 slot columns
    eb = np.exp(rel_emb[NUM_BUCKETS - 1, :].astype(np.float32))  # [H]
    ebrow_all = np.broadcast_to(
        np.repeat(eb, 65)[None, :], (128, H * 65)
    ).astype(np.float16)

    in_maps = []
    for core in range(NCORES):
        b, hg = core // 2, core % 2
        csl = slice(hg * CL, (hg + 1) * CL)
        in_maps.append(
            {
                "xT": np.ascontiguousarray(x[b].T).astype(np.float16),
                "wq": np.ascontiguousarray(W_attn[csl, :].T).astype(np.float16),
                "wk": np.ascontiguousarray(W_attn[C + hg * CL : C + (hg + 1) * CL, :].T).astype(np.float16),
                "wv": np.ascontiguousarray(W_attn[2 * C + hg * CL : 2 * C + (hg + 1) * CL, :].T).astype(np.float16),
                "wp": np.ascontiguousarray(W_proj[:, csl].T).astype(np.float16),
                "wexp": np.ascontiguousarray(wexp_all[hg * HL : (hg + 1) * HL]),
                "ebrow": np.ascontiguousarray(
                    ebrow_all[:, hg * HL * 65 : (hg + 1) * HL * 65]
                ),
            }
        )
    return in_maps


def kernel(x, W_attn, b_attn, W_proj, b_proj, rel_emb):
    x = np.asarray(x)
    W_attn = np.asarray(W_attn)
    b_attn = np.asarray(b_attn)
    W_proj = np.asarray(W_proj)
    b_proj = np.asarray(b_proj)
    rel_emb = np.asarray(rel_emb)
    assert not b_attn.any(), "kernel specialised for zero attn bias"

    in_maps = _make_in_maps(x, W_attn, b_attn, W_proj, rel_emb)
    nc = _get_program()
    res = bass_utils.run_bass_kernel_spmd(nc, in_maps, core_ids=list(range(NCORES)))
    global LAST_RESULTS
    LAST_RESULTS = res

    y = np.empty((B, T, C), dtype=np.float32)
    for b in range(B):
        ypT = res.results[2 * b]["yp"].astype(np.float32) + res.results[2 * b + 1][
            "yp"
        ].astype(np.float32)
        y[b] = ypT.T + b_proj[None, :].astype(np.float32)
    return y


# revision 24
# speedup vs baseline: 1.0242x; 1.0078x over previous
"""Trainium2 Bass kernel for causal self-attention with T5 relative position bias.

Problem (hardcoded): B=4, T=2048, C=1024, H=16, D=64, NUM_BUCKETS=32, MAX_DISTANCE=128.
Sharding over 8 cores: core c -> (batch b=c//2, head-group hg=c%2 of 8 heads).
Each core computes qkv projection for its heads, causal attention, and a partial
output projection (its heads' rows of W_proj); host sums the two partials per batch.

v4 structure (baseline 409us -> v2 381 -> v3 ~325 -> this):
  - One software-pipelined stream of attention "superpairs": chunk-outer,
    head-PAIR inner.  The even/odd heads of a pair sit at partitions 0-63 /
    64-127, so their K=64 S matmuls land on disjoint PE row groups and run
    CONCURRENTLY (tile_position auto-derived from the base partition;
    HW-verified 216ns wall for both) -- 2x S throughput vs head-serial.
  - Pair t+1's S matmuls + EXP are emitted before pair t's AV matmuls so the
    PE never head-of-line blocks on the exp/mask chain.
  - Stage-1 (qkv) and stage-3 (proj) matmul groups are emitted as FILLER
    between pairs, paced with a terminal reserve so the PE never starves and
    the HAM clock gate keeps the array at 2.4 GHz (the phase-serial baseline
    ran most S/AV matmuls at 1.2 GHz).
  - T5 bias for far tiles (min distance >= 113: constant bucket 31) is folded
    into a second V table vf = v * exp(b31): far tiles use raw exp(S/8) with
    vf, near tiles use the ea mask/bias table with v.  No vector mask-multiply
    and no ea read for ~53% of elements; the ea table shrinks to its live
    window [384:1152) of the Toeplitz layout.
  - Softmax denominators: rowsum rows bounce through DRAM for the partition
    broadcast (a partition-strided DMA dst writes garbage on HW -- use row
    DMAs); reciprocals batch per (chunk, half) on the DVE.  The very last
    head pair is normalized via a K=1 fp16 broadcast matmul instead (PE queue
    is empty at the tail, and it skips the DRAM bounce latency).
    (reciprocal_approx_fast was tried: broken on HW, returns garbage.)
  - b_attn is always zero for this problem: evacuations are plain copies and
    the v-scatter is one strided copy per t16 tile.
  - Partial outputs yp in fp16 (host sums the two cores' partials in fp32).

On-chip layout (from baseline):
  - x, q, k transposed ([C, T]-style) so matmuls contract over partitions.
  - S computed transposed: S_T[tk, tq] = k_h^T q_h (K=64); softmax skips
    max-subtraction (logits ~ N(0,1)).
  - Superpair PSUM tile [128,1024] = [head A (512) | head B (512)], one EXP
    over both heads via a two-window strided AP.
  - Row sums from a ones-column appended to V (AV matmul M=65).
"""

import sys

sys.path.insert(0, "/opt/trn_rl_repo")

import math

import numpy as np

import concourse.bacc as bacc
import concourse.bass as bass
import concourse.mybir as mybir
import concourse.tile as tile
from concourse import bass_utils


def _ensure_axon_hooks():
    try:
        import antenv.axon_hooks  # noqa: F401
    except Exception:
        try:
            import types

            import antenv

            hooks = types.ModuleType("antenv.axon_hooks")
            hooks._hook = None
            hooks.set_axon_ntff_profile_hook = lambda h: setattr(hooks, "_hook", h)
            hooks.get_axon_ntff_profile_hook = lambda: hooks._hook
            sys.modules["antenv.axon_hooks"] = hooks
            antenv.axon_hooks = hooks
        except Exception:
            pass


_ensure_axon_hooks()

B, T, C = 4, 2048, 1024
H, D = 16, 64
NUM_BUCKETS, MAX_DISTANCE = 32, 128
HL = 8  # local heads per core
CL = HL * D  # 512 local channels
NCORES = 8

FP16 = mybir.dt.float16
FP32 = mybir.dt.float32

# ea table: live window only.  Near tiles (j >= 4c-1) read s_off in
# {384, 512} with width <= 640, so columns [384, 1152) of the full Toeplitz
# layout suffice.  EA_BASE is subtracted from s_off at use.
EA_W = 768
EA_BASE = 384
FAR_CUT = 113  # bucket(d) == 31 for all d >= 113 (fp32-exact)

NT = T // 512  # 4 tq chunks of 512
NK = T // 128  # 16 tk tiles of 128
KC = C // 128  # 8 contraction chunks for qkv
MC = CL // 128  # 4 m-chunks of local channels

# filler drain rate (groups per superpair) per chunk
DRAIN_RATE = [0.8, 0.5, 0.45, 0.4]


def _build_program():
    nc = bacc.Bacc(None, target_bir_lowering=False)

    xT = nc.dram_tensor("xT", [C, T], FP16, kind="ExternalInput")
    wq = nc.dram_tensor("wq", [C, CL], FP16, kind="ExternalInput")
    wk = nc.dram_tensor("wk", [C, CL], FP16, kind="ExternalInput")
    wv = nc.dram_tensor("wv", [C, CL], FP16, kind="ExternalInput")
    wp = nc.dram_tensor("wp", [CL, C], FP16, kind="ExternalInput")
    wexp = nc.dram_tensor("wexp", [HL, 128, EA_W], FP16, kind="ExternalInput")
    ebrow = nc.dram_tensor("ebrow", [128, HL * 65], FP16, kind="ExternalInput")
    yp = nc.dram_tensor("yp", [C, T], FP16, kind="ExternalOutput")
    rscratch = nc.dram_tensor("rscratch", [NT * HL, 512], FP16)

    with tile.TileContext(nc) as tc:
        with (
            tc.tile_pool(name="w", bufs=1) as wpool,
            tc.tile_pool(name="big", bufs=1) as bigpool,
            tc.tile_pool(name="tr", bufs=3) as tr,
            tc.tile_pool(name="sm", bufs=2) as smpool,
            tc.tile_pool(name="ev", bufs=7) as evpool,
            tc.tile_pool(name="ps", bufs=2, space="PSUM") as poolS,
            tc.tile_pool(name="pav", bufs=2, space="PSUM") as poolAV,
            tc.tile_pool(name="pf", bufs=2, space="PSUM") as poolF,
        ):
            # ---- weights / constants ----
            wq_sb = wpool.tile([128, KC, CL], FP16)
            wk_sb = wpool.tile([128, KC, CL], FP16)
            wv_sb = wpool.tile([128, KC, CL], FP16)
            wp_sb = wpool.tile([128, MC, C], FP16)
            ea_sb = wpool.tile([128, HL, EA_W], FP16)
            eb_sb = wpool.tile([128, HL * 65], FP16)
            ones_bc = wpool.tile([97, 64], FP16)
            nc.vector.memset(ones_bc[:], 1.0)
            xt_sb = bigpool.tile([128, KC, T], FP16)
            xr = xT.rearrange("(kc p) (tc t) -> p kc tc t", p=128, t=512)

            nc.sync.dma_start(out=eb_sb, in_=ebrow[:])
            wq_r = wq.rearrange("(kc p) m -> p kc m", p=128)
            wk_r = wk.rearrange("(kc p) m -> p kc m", p=128)
            wv_r = wv.rearrange("(kc p) m -> p kc m", p=128)
            for kc in range(KC):
                nc.sync.dma_start(out=xt_sb[:, kc, 0:512], in_=xr[:, kc, 0])
                nc.sync.dma_start(out=wq_sb[:, kc], in_=wq_r[:, kc])
                nc.sync.dma_start(out=wk_sb[:, kc], in_=wk_r[:, kc])
                nc.sync.dma_start(out=wv_sb[:, kc], in_=wv_r[:, kc])
            for l in range(HL):
                nc.sync.dma_start(out=ea_sb[:, l], in_=wexp[l])
            for tch in range(1, NT):
                nc.sync.dma_start(
                    out=xt_sb[:, :, tch * 512 : (tch + 1) * 512], in_=xr[:, :, tch]
                )
            nc.sync.dma_start(out=wp_sb, in_=wp.rearrange("(kc p) m -> p kc m", p=128))

            # ---- persistent activations ----
            qT_sb = bigpool.tile([128, MC, T], FP16)
            kT_sb = bigpool.tile([128, MC, T], FP16)
            v_sb = bigpool.tile([128, NK, HL * 65], FP16)  # slot l: [v(64), ones]
            vf_sb = bigpool.tile([128, NK, HL * 65], FP16)  # v * exp(b31), far tiles
            y_sb = bigpool.tile([128, MC, T], FP16)

            for l in range(HL):
                nc.vector.memset(v_sb[:, :, l * 65 + 64 : l * 65 + 65], 1.0)

            # ---- stage-1 / stage-3 matmul groups (emitted as filler) ----
            def emit_qk(which, m, tch):
                w_sb, dst = (wq_sb, qT_sb) if which == "q" else (wk_sb, kT_sb)
                tsl = slice(tch * 512, (tch + 1) * 512)
                msl = slice(m * 128, (m + 1) * 128)
                p = poolF.tile([128, 512], FP32, tag="pf")
                for kc in range(KC):
                    nc.tensor.matmul(
                        p[:], w_sb[:, kc, msl], xt_sb[:, kc, tsl],
                        start=(kc == 0), stop=(kc == KC - 1),
                    )
                nc.vector.tensor_copy(dst[:, m, tsl], p[:])

            def emit_v(t16):
                p = poolF.tile([128, 512], FP32, tag="pf")
                for kc in range(KC):
                    nc.tensor.matmul(
                        p[:], xt_sb[:, kc, t16 * 128 : (t16 + 1) * 128], wv_sb[:, kc, :],
                        start=(kc == 0), stop=(kc == KC - 1),
                    )
                base = v_sb[:, t16]
                dst = bass.AP(
                    tensor=base.tensor, offset=base.offset,
                    ap=[base.ap[0], [65, 8], [1, 64]],
                )
                src = bass.AP(
                    tensor=p.tensor, offset=p.offset,
                    ap=[p.ap[0], [64, 8], [1, 64]],
                )
                nc.vector.tensor_copy(dst, src)
                # far-tile table: v * exp(b31) columnwise (ones col becomes
                # exp(b31), giving bias-consistent rowsums)
                nc.vector.tensor_mul(
                    out=vf_sb[:, t16, :], in0=v_sb[:, t16, :], in1=eb_sb[:],
                )

            def emit_proj(tch, mo):
                tsl = slice(tch * 512, (tch + 1) * 512)
                osl = slice(mo * 128, (mo + 1) * 128)
                p = poolF.tile([128, 512], FP32, tag="pf")
                for kcm in range(MC):
                    nc.tensor.matmul(
                        p[:], wp_sb[:, kcm, osl], y_sb[:, kcm, tsl],
                        start=(kcm == 0), stop=(kcm == MC - 1),
                    )
                yo = tr.tile([128, 512], FP16, tag="yo")
                nc.vector.tensor_copy(yo[:], p[:])
                nc.sync.dma_start(out=yp[osl, tsl], in_=yo[:])

            # filler queue, dependency-safe order; proj groups appended later
            fillers = []  # (key, thunk)
            for tch in range(NT):
                for t16 in range(4 * tch, 4 * tch + 4):
                    fillers.append((("v", t16), lambda t16=t16: emit_v(t16)))
                for m in range(MC):
                    fillers.append((("q", m, tch), lambda m=m, tch=tch: emit_qk("q", m, tch)))
                    fillers.append((("k", m, tch), lambda m=m, tch=tch: emit_qk("k", m, tch)))

            emitted_keys = set()
            state = {"credit": 0.0, "step": 0}

            def drain_one():
                key, thunk = fillers.pop(0)
                thunk()
                emitted_keys.add(key)

            def need(*keys):
                while fillers and not all(k in emitted_keys for k in keys):
                    drain_one()

            def drain_credit(rate):
                state["credit"] += rate
                # keep a filler reserve so the PE never starves near the end
                floor = max(0, (NSTEP - state["step"]) // 8)
                while state["credit"] >= 1.0 and len(fillers) > floor:
                    drain_one()
                    state["credit"] -= 1.0

            deferred = []  # (eligible_step, closure): normalize work is held
            # back so its upstream recip/DMA chain is done before anything it
            # emits can reach an engine queue and head-of-line stall it

            def drain_deferred(n=1):
                while n > 0 and deferred and deferred[0][0] <= state["step"]:
                    deferred.pop(0)[1]()
                    n -= 1

            # ---- the attention stream: chunk-outer, head-pair inner ----
            steps = []
            for c in range(NT):
                for hp in range(4):
                    for j in range(4 * c + 4):
                        steps.append((c, hp, j))
            NSTEP = len(steps)

            ctx = {}
            pav_tiles = {}
            rsg_tiles = {}
            yev_tiles = {}

            def emit_S(t):
                c, hp, j = steps[t]
                if j == 0:
                    if hp == 0:
                        need(("v", 4 * c + 3))
                    need(("q", hp, c), ("k", hp, c))
                off = max(0, 128 * j - 512 * c)
                far = (512 * c - 128 * j - 127) >= FAR_CUT
                w = 512 - off
                pS = poolS.tile([128, 1024], FP32, tag="pS")
                # the two heads' K=64 matmuls occupy disjoint PE row groups
                # (tile_position from base partition) and run concurrently
                nc.tensor.matmul(
                    pS[:, off:512],
                    kT_sb[0:64, hp, j * 128 : (j + 1) * 128],
                    qT_sb[0:64, hp, c * 512 + off : (c + 1) * 512],
                    start=True, stop=True,
                )
                nc.tensor.matmul(
                    pS[:, 512 + off : 1024],
                    kT_sb[64:128, hp, j * 128 : (j + 1) * 128],
                    qT_sb[64:128, hp, c * 512 + off : (c + 1) * 512],
                    start=True, stop=True,
                )

                def win(ap_t):
                    return bass.AP(
                        tensor=ap_t.tensor, offset=ap_t.offset + off,
                        ap=[ap_t.ap[0], [512, 2], [1, w]],
                    )

                pm2 = tr.tile([128, 1024], FP16, tag="pm")
                if far:
                    nc.scalar.activation(
                        out=win(pm2), in_=win(pS),
                        func=mybir.ActivationFunctionType.Exp,
                        scale=1.0 / math.sqrt(D),
                    )
                else:
                    p2 = tr.tile([128, 1024], FP16, tag="p")
                    nc.scalar.activation(
                        out=win(p2), in_=win(pS),
                        func=mybir.ActivationFunctionType.Exp,
                        scale=1.0 / math.sqrt(D),
                    )
                    s_off = 512 * c - 128 * j + 384 + off
                    ea_h = ea_sb[:, 2 * hp]
                    ea_ap = bass.AP(
                        tensor=ea_h.tensor,
                        offset=ea_h.offset + (s_off - EA_BASE),
                        ap=[ea_h.ap[0], [EA_W, 2], [1, w]],
                    )
                    nc.vector.tensor_mul(out=win(pm2), in0=win(p2), in1=ea_ap)
                ctx[t] = (c, hp, j, pm2, off, far)

            def emit_AV(t):
                c, hp, j, pm2, off, far = ctx.pop(t)
                nj = 4 * c + 4
                vtab = vf_sb if far else v_sb
                for h in range(2):
                    l = 2 * hp + h
                    if j == 0:
                        pav_tiles[(c, l)] = poolAV.tile(
                            [65, 512], FP32, tag="pav", name="pav"
                        )
                    pav = pav_tiles[(c, l)]
                    nc.tensor.matmul(
                        pav[:, off:512],
                        vtab[:, j, l * 65 : l * 65 + 65],
                        pm2[:, 512 * h + off : 512 * h + 512],
                        start=(j == 0),
                        stop=(j == nj - 1),
                    )
                if j == nj - 1:
                    for h in range(2):
                        post_head_chunk(c, 2 * hp + h)

            def post_head_chunk(c, l):
                pav = pav_tiles.pop((c, l))
                yev = evpool.tile([128, 512], FP32, tag="yev")
                nc.vector.tensor_copy(yev[0:65, :], pav[0:65, :])
                yev_tiles[(c, l)] = yev
                if c == NT - 1 and l >= 6:
                    tail_norm(l)
                    return
                if c == NT - 1 and l >= 4:
                    # chunk 3 heads 4,5: finish early so the recip runs during
                    # the final head pair
                    if l == 4:
                        rsg_tiles[(c, 1)] = smpool.tile(
                            [4, 512], FP32, tag="rsg", name="rsg32"
                        )
                    nc.sync.dma_start(
                        out=rsg_tiles[(c, 1)][l - 4 : l - 3, :], in_=yev[64:65, :]
                    )
                    if l == 5:
                        finish_half(c, 1, nrows=2)
                    return
                half = l // 4
                if l % 4 == 0:
                    rsg_tiles[(c, half)] = smpool.tile(
                        [4, 512], FP32, tag="rsg", name="rsg32"
                    )
                nc.sync.dma_start(
                    out=rsg_tiles[(c, half)][l % 4 : l % 4 + 1, :], in_=yev[64:65, :]
                )
                if l % 4 == 3:
                    finish_half(c, half)

            def finish_half(c, half, nrows=4):
                # reciprocal cost is ~6.4ns per free element regardless of
                # partition count; emit it in 128-column pieces drained one
                # per superpair so the DVE queue never blocks ~3.3us at once
                rsg = rsg_tiles.pop((c, half))
                rec32 = smpool.tile([4, 512], FP32, tag="rec32")
                rec16 = smpool.tile([4, 512], FP16, tag="rec16")
                s0 = state["step"]

                def piece(q):
                    cs = slice(q * 128, (q + 1) * 128)
                    nc.vector.reciprocal(out=rec32[0:nrows, cs], in_=rsg[0:nrows, cs])

                def flush():
                    nc.vector.tensor_copy(rec16[0:nrows, :], rec32[0:nrows, :])
                    r0 = c * HL + half * 4
                    nc.sync.dma_start(
                        out=rscratch[r0 : r0 + nrows], in_=rec16[0:nrows, :]
                    )

                for q in range(4):
                    deferred.append((s0 + 1 + q, lambda q=q: piece(q)))
                deferred.append((s0 + 5, lambda: flush()))
                elig = s0 + 7
                for l in range(half * 4, half * 4 + nrows):
                    deferred.append(
                        (elig, lambda c=c, l=l: normalize_bounce(c, l))
                    )
                if half == 1 and nrows == 4:
                    deferred.append((elig, lambda c=c: push_proj(c)))

            tail_state = {}

            def tail_norm(l):
                # heads 6,7 of the last chunk: batched recip + in-SBUF
                # broadcast via K=1 fp16 matmuls (PE queue is empty here)
                c = NT - 1
                if l == 6:
                    trsg = smpool.tile([2, 512], FP32, tag="rsg", name="trsg")
                    tail_state["trsg"] = trsg
                    nc.sync.dma_start(out=trsg[0:1, :], in_=yev_tiles[(c, 6)][64:65, :])
                    return
                trsg = tail_state["trsg"]
                nc.sync.dma_start(out=trsg[1:2, :], in_=yev_tiles[(c, 7)][64:65, :])
                while fillers:
                    drain_one()  # reserved groups keep the PE warm here
                trec = smpool.tile([33, 512], FP32, tag="rec32", name="trec")
                nc.vector.reciprocal(out=trec[0:2, :], in_=trsg[0:2, :])
                trec16 = smpool.tile([33, 512], FP16, tag="rec16", name="trec16")
                nc.vector.tensor_copy(trec16[0:2, :], trec[0:2, :])
                nc.sync.dma_start(out=trec16[32:33, :], in_=trec16[1:2, :])
                normalize(c, 6, trec16[0:1, :], bp=0)
                normalize(c, 7, trec16[32:33, :], bp=32)
                push_proj(c)

            def push_proj(c):
                for mo in range(C // 128):
                    fillers.append((("proj", c, mo), lambda c=c, mo=mo: emit_proj(c, mo)))

            def normalize_bounce(c, l):
                srow = rscratch[c * HL + l]
                bc_sb = smpool.tile([64, 512], FP16, tag="bcsb")
                nc.sync.dma_start(
                    out=bc_sb[:],
                    in_=bass.AP(
                        tensor=srow.tensor, offset=srow.offset,
                        ap=[[0, 64], [1, 512]],
                    ),
                )
                finish_norm(c, l, bc_sb[:])

            def normalize(c, l, rrow, bp=0):
                bcp = poolF.tile([64, 512], FP32, tag="pf", name="bcp")
                nc.tensor.matmul(
                    bcp[:], ones_bc[bp : bp + 1, :], rrow, start=True, stop=True
                )
                finish_norm(c, l, bcp[:])

            def finish_norm(c, l, scale_ap):
                mq = l // 2
                fullq = slice(c * 512, (c + 1) * 512)
                yev = yev_tiles.pop((c, l))
                if l % 2 == 0:
                    nc.vector.tensor_mul(
                        out=y_sb[0:64, mq, fullq], in0=yev[0:64, :], in1=scale_ap,
                    )
                else:
                    ytmp = smpool.tile([64, 512], FP16, tag="ytmp")
                    nc.vector.tensor_mul(out=ytmp[:], in0=yev[0:64, :], in1=scale_ap)
                    nc.sync.dma_start(out=y_sb[64:128, mq, fullq], in_=ytmp[:])

            # ---- run the pipelined stream ----
            emit_S(0)
            for t in range(1, NSTEP):
                state["step"] = t
                emit_S(t)
                emit_AV(t - 1)
                drain_deferred(1)
                drain_credit(DRAIN_RATE[steps[t][0]])
            state["step"] = NSTEP + 99
            emit_AV(NSTEP - 1)
            drain_deferred(99)
            while fillers:
                drain_one()

    nc.compile()
    return nc


_NC = None
LAST_RESULTS = None


def _get_program():
    global _NC
    if _NC is None:
        _NC = _build_program()
    return _NC


def _rel_bias_buckets():
    """bucket(d) for d = q - k in [0, T): exact float32 replica of the reference."""
    d = np.arange(T)
    max_exact = NUM_BUCKETS // 2
    rpf = d.astype(np.float32) / np.float32(max_exact) + np.float32(1e-10)
    val = (
        np.log(rpf)
        / np.float32(math.log(MAX_DISTANCE / max_exact))
        * np.float32(NUM_BUCKETS - max_exact)
    )
    large = max_exact + val.astype(np.int32)
    large = np.minimum(large, NUM_BUCKETS - 1)
    return np.where(d < max_exact, d, large)


def _make_in_maps(x, W_attn, b_attn, W_proj, rel_emb):
    buckets = _rel_bias_buckets()  # [T]
    assert buckets[FAR_CUT - 1] < NUM_BUCKETS - 1
    assert buckets[FAR_CUT] == NUM_BUCKETS - 1
    bias_by_dist = rel_emb[buckets, :]  # [T, H] fp32
    # vec[h, j] = exp(bias[j - 511]) for j >= 511 else 0   (j - 511 = distance d)
    vec = np.zeros((H, 2432 + 127), dtype=np.float32)
    vec[:, 511 : 511 + T] = np.exp(bias_by_dist.T)
    vec = vec.astype(np.float16)
    # per-head Toeplitz table A[h, p, x] = vec[h, x - p + 127], live window only
    sw = np.lib.stride_tricks.sliding_window_view(vec, 2432, axis=1)  # [H, 128, 2432]
    wexp_all = np.ascontiguousarray(sw[:, ::-1, EA_BASE : EA_BASE + EA_W])

    # ebrow: exp(b31) per local head, replicated over the 65 slot columns
    eb = np.exp(rel_emb[NUM_BUCKETS - 1, :].astype(np.float32))  # [H]
    ebrow_all = np.broadcast_to(
        np.repeat(eb, 65)[None, :], (128, H * 65)
    ).astype(np.float16)

    in_maps = []
    for core in range(NCORES):
        b, hg = core // 2, core % 2
        csl = slice(hg * CL, (hg + 1) * CL)
        in_maps.append(
            {
                "xT": np.ascontiguousarray(x[b].T).astype(np.float16),
                "wq": np.ascontiguousarray(W_attn[csl, :].T).astype(np.float16),
                "wk": np.ascontiguousarray(W_attn[C + hg * CL : C + (hg + 1) * CL, :].T).astype(np.float16),
                "wv": np.ascontiguousarray(W_attn[2 * C + hg * CL : 2 * C + (hg + 1) * CL, :].T).astype(np.float16),
                "wp": np.ascontiguousarray(W_proj[:, csl].T).astype(np.float16),
                "wexp": np.ascontiguousarray(wexp_all[hg * HL : (hg + 1) * HL]),
                "ebrow": np.ascontiguousarray(
                    ebrow_all[:, hg * HL * 65 : (hg + 1) * HL * 65]
                ),
            }
        )
    return in_maps


def kernel(x, W_attn, b_attn, W_proj, b_proj, rel_emb):
    x = np.asarray(x)
    W_attn = np.asarray(W_attn)
    b_attn = np.asarray(b_attn)
    W_proj = np.asarray(W_proj)
    b_proj = np.asarray(b_proj)
    rel_emb = np.asarray(rel_emb)
    assert not b_attn.any(), "kernel specialised for zero attn bias"

    in_maps = _make_in_maps(x, W_attn, b_attn, W_proj, rel_emb)
    nc = _get_program()
    res = bass_utils.run_bass_kernel_spmd(nc, in_maps, core_ids=list(range(NCORES)))
    global LAST_RESULTS
    LAST_RESULTS = res

    y = np.empty((B, T, C), dtype=np.float32)
    for b in range(B):
        ypT = res.results[2 * b]["yp"].astype(np.float32) + res.results[2 * b + 1][
            "yp"
        ].astype(np.float32)
        y[b] = ypT.T + b_proj[None, :].astype(np.float32)
    return y
